# revision 19
# baseline (speedup 1.0000x reference)
"""Multi-head attention (B=4, S=2048, D=1024, H=16, HS=64, causal) on 8 trn2 cores.

Strategy: tensor-parallel over heads (2 heads per core), x replicated.
Per core: Q/K projections (bf16 matmuls, N=512 moving), V projected
directly into [token, vdim] layout via x-stationary matmuls (no PE
transposes), causal attention with transposed-scores softmax (no
max-subtraction; scores are O(1) by construction) at 512-wide chunks,
output projection of the local head pair -> partial [B*S, D] in bf16.
Host sums the 8 partials (the Wo contraction over heads).

All matmul operands are bf16 (PSUM accumulation stays fp32); numpy
simulation of this pipeline gives max-rel-err ~4.4e-3 vs the fp32
reference (gate is 2e-2). bf16 halves DMA traffic and allows wide
moving streams at 1 cycle/row. Wide chunks keep the per-rep
instruction count low (HW pays ~30ns/instruction over the cost model).
"""

import sys

sys.path.insert(0, "/opt/trn_rl_repo")

import numpy as np
import ml_dtypes

import concourse.bacc as bacc
import concourse.bass as bass
import concourse.mybir as mybir
import concourse.tile as tile
from concourse.bass_utils import run_bass_kernel_spmd

F32 = mybir.dt.float32
F32R = mybir.dt.float32r
BF16 = mybir.dt.bfloat16
EXP = mybir.ActivationFunctionType.Exp
MUL = mybir.AluOpType.mult
ADD = mybir.AluOpType.add
NPBF = ml_dtypes.bfloat16

B, S, D, H, HS = 4, 2048, 1024, 16, 64
ROWS = B * S                      # 8192
NB = 8                            # cores
SCALE = 1.0 / float(np.sqrt(HS))  # 0.125
CH = 512                          # scores/AV moving width (q positions)
QKCH = 512                       # q/k projection moving width (psum bank cap)
NC_CH = S // CH                   # 4 q chunks per batch
NKT = S // 128                    # 16 k-tiles per batch
NMASK = CH // 128                 # 4 diagonal mask tiles

TRACE = False
LAST_RESULTS = None
LAST_IN_MAPS = None


def _build_nc(reps: int = 1):
    nc = bacc.Bacc()

    xT = nc.declare_dram_parameter("xT", [D, ROWS], BF16, isOutput=False)
    wq = nc.declare_dram_parameter("wq", [D, 128], BF16, isOutput=False)
    wk = nc.declare_dram_parameter("wk", [D, 128], BF16, isOutput=False)
    wv = nc.declare_dram_parameter("wv", [D, 128], BF16, isOutput=False)
    wo0 = nc.declare_dram_parameter("wo0", [128, D], BF16, isOutput=False)
    wo1 = nc.declare_dram_parameter("wo1", [128, D], BF16, isOutput=False)
    bq = nc.declare_dram_parameter("bq", [128, 1], F32, isOutput=False)
    bk = nc.declare_dram_parameter("bk", [128, 1], F32, isOutput=False)
    bv = nc.declare_dram_parameter("bv", [128, 128], BF16, isOutput=False)
    masks = nc.declare_dram_parameter("masks", [128, NMASK * CH], BF16,
                                      isOutput=False)
    ones = nc.declare_dram_parameter("ones", [128, 128], F32R, isOutput=False)
    out = nc.declare_dram_parameter("out", [ROWS, D], BF16, isOutput=True)

    with tile.TileContext(nc) as tc:
        with tc.tile_pool(name="const", bufs=1) as cpool, \
             tc.tile_pool(name="sb", bufs=1) as sb, \
             tc.tile_pool(name="ps", bufs=1, space="PSUM") as ps:

            # ---- persistent constants ----
            wq_sb = cpool.tile([128, 8 * 128], BF16)
            wk_sb = cpool.tile([128, 8 * 128], BF16)
            wv_sb = cpool.tile([128, 8 * 128], BF16)
            for w_sb, w in ((wq_sb, wq), (wk_sb, wk), (wv_sb, wv)):
                nc.sync.dma_start(
                    w_sb[:, :].rearrange("p (dk j) -> p dk j", dk=8),
                    w[:, :].rearrange("(dk p) j -> p dk j", dk=8),
                )
            wo0_sb = cpool.tile([128, D], BF16)
            wo1_sb = cpool.tile([128, D], BF16)
            nc.sync.dma_start(wo0_sb[:, :], wo0[:, :])
            nc.sync.dma_start(wo1_sb[:, :], wo1[:, :])
            bq_sb = cpool.tile([128, 1], F32)
            bk_sb = cpool.tile([128, 1], F32)
            bv_sb = cpool.tile([128, 128], BF16)
            nc.sync.dma_start(bq_sb[:, :], bq[:, :])
            nc.sync.dma_start(bk_sb[:, :], bk[:, :])
            nc.sync.dma_start(bv_sb[:, :], bv[:, :])
            masks_sb = cpool.tile([128, NMASK * CH], BF16)
            nc.sync.dma_start(masks_sb[:, :], masks[:, :])
            ones_sb = cpool.tile([128, 128], F32R)
            nc.sync.dma_start(ones_sb[:, :], ones[:, :])

            # persistent zero-padded per-head tensors, double-buffered
            qz = [[cpool.tile([128, S], BF16, name=f"qz{h}{p}")
                   for p in range(2)] for h in range(2)]   # [h][parity]
            kz = [[cpool.tile([128, S], BF16, name=f"kz{h}{p}")
                   for p in range(2)] for h in range(2)]
            oz = [[cpool.tile([128, S], BF16, name=f"oz{h}{p}")
                   for p in range(2)] for h in range(2)]
            v1z = [[cpool.tile([128, NKT * 128], BF16, name=f"v1z{h}{p}")
                    for p in range(2)] for h in range(2)]
            for grp in (qz, kz, oz, v1z):
                for h in range(2):
                    for par in range(2):
                        nc.vector.memset(grp[h][par][:, :], 0.0)
            for h in range(2):
                for par in range(2):
                    v1 = v1z[h][par]
                    ones_dst = bass.AP(
                        v1.tensor, v1.offset + 64,
                        [v1.ap[0]] + [[128, NKT]])
                    nc.vector.tensor_copy(ones_dst, ones_sb[:, 0:NKT])

            env = dict(
                wq_sb=wq_sb, wk_sb=wk_sb, wv_sb=wv_sb, wo0_sb=wo0_sb,
                wo1_sb=wo1_sb, bq_sb=bq_sb, bk_sb=bk_sb, bv_sb=bv_sb,
                masks_sb=masks_sb, ones_sb=ones_sb, xT=xT, out=out,
                qz=qz, kz=kz, oz=oz, v1z=v1z)

            if reps > 1:
                with tc.For_i(0, reps, 1):
                    _emit(nc, tc, sb, ps, env)
            else:
                _emit(nc, tc, sb, ps, env)
    nc.compile()
    return nc


def _emit(nc, tc, sb, ps, env):
    wq_sb = env["wq_sb"]; wk_sb = env["wk_sb"]; wv_sb = env["wv_sb"]
    wo0_sb = env["wo0_sb"]; wo1_sb = env["wo1_sb"]
    bq_sb = env["bq_sb"]; bk_sb = env["bk_sb"]; bv_sb = env["bv_sb"]
    masks_sb = env["masks_sb"]; ones_sb = env["ones_sb"]
    xT = env["xT"]; out = env["out"]
    qz = env["qz"]; kz = env["kz"]; oz = env["oz"]; v1z = env["v1z"]

    st_ = {}   # per-b tiles

    def qkv_unit(b, cp):
        def run():
            r0 = b * S
            par = b % 2
            xt = sb.tile([128, 8 * QKCH], BF16, tag="xt", bufs=2,
                         name=f"xt{b}_{cp}")
            nc.sync.dma_start(
                xt[:, :].rearrange("p (dk j) -> p dk j", dk=8),
                xT[:, r0 + cp * QKCH: r0 + (cp + 1) * QKCH]
                .rearrange("(dk p) j -> p dk j", dk=8),
            )
            cs = cp * QKCH
            for w_sb, dz, bias in ((wq_sb, qz, bq_sb), (wk_sb, kz, bk_sb)):
                pp = ps.tile([128, QKCH], F32, tag="st", bufs=2,
                             name=f"pp{b}{cp}")
                for dk in range(8):
                    nc.tensor.matmul(
                        pp[:, :],
                        w_sb[:, dk * 128:(dk + 1) * 128],
                        xt[:, dk * QKCH:(dk + 1) * QKCH],
                        start=(dk == 0), stop=(dk == 7),
                    )
                nc.vector.tensor_scalar_add(
                    dz[0][par][0:64, cs:cs + QKCH], pp[0:64, :],
                    bias[0:64, :])
                nc.vector.tensor_scalar_add(
                    dz[1][par][64:128, cs:cs + QKCH], pp[64:128, :],
                    bias[64:128, :])
            # V directly in [token, vdim] layout: x chunk stationary,
            # Wv moving (both heads at once).
            for qt in range(QKCH // 128):
                pv = ps.tile([128, 128], F32, tag="bcx", bufs=2,
                             name=f"pv{b}{cp}{qt}")
                for dk in range(8):
                    nc.tensor.matmul(
                        pv[:, :],
                        xt[:, dk * QKCH + qt * 128: dk * QKCH + (qt + 1) * 128],
                        wv_sb[:, dk * 128:(dk + 1) * 128],
                        start=(dk == 0), stop=(dk == 7),
                    )
                kt = cp * (QKCH // 128) + qt
                for h in range(2):
                    nc.vector.tensor_tensor(
                        v1z[h][par][:, kt * 128: kt * 128 + 64],
                        pv[:, h * 64:(h + 1) * 64],
                        bv_sb[:, h * 64:(h + 1) * 64],
                        ADD,
                    )
        return run

    def attn_unit(b, c, h):
        def run():
            par = b % 2
            qTz, kTz, oTz = qz[h][par], kz[h][par], oz[h][par]
            v1 = v1z[h][par]
            qs = c * CH
            nk = NMASK * (c + 1)          # k-tiles for this chunk
            po = ps.tile([128, CH], F32, tag="po", bufs=2, name=f"po{b}{h}{c}")
            for g in range((nk + 1) // 2):
                k0 = 2 * g
                gn = min(2, nk - k0)
                stp = ps.tile([128, 1024], F32, tag="st", bufs=2,
                              name=f"stp{b}{h}{c}{g}")
                for j in range(gn):
                    nc.tensor.matmul(
                        stp[:, j * CH:(j + 1) * CH],
                        kTz[:, (k0 + j) * 128:(k0 + j + 1) * 128],
                        qTz[:, qs:qs + CH],
                        start=True, stop=True,
                    )
                pt = sb.tile([128, 1024], BF16, tag="pt", bufs=8,
                             name=f"pt{b}{h}{c}{g}")
                nc.scalar.activation(pt[:, 0:gn * CH], stp[:, 0:gn * CH],
                                     EXP, scale=SCALE)
                for j in range(gn):
                    m = (k0 + j) - NMASK * c
                    if m >= 0:
                        # cols [0, 128m) of this tile are fully causal-masked:
                        # skip them here and in the AV stream below.
                        lo = 128 * m
                        nc.vector.tensor_tensor(
                            pt[:, j * CH + lo:(j + 1) * CH],
                            pt[:, j * CH + lo:(j + 1) * CH],
                            masks_sb[:, m * CH + lo:(m + 1) * CH],
                            MUL,
                        )
                for j in range(gn):
                    kt = k0 + j
                    m = kt - NMASK * c
                    lo = 128 * m if m > 0 else 0
                    nc.tensor.matmul(
                        po[:, lo:],
                        v1[:, kt * 128:(kt + 1) * 128],
                        pt[:, j * CH + lo:(j + 1) * CH],
                        start=(kt == 0), stop=(kt == nk - 1),
                    )
            rec = sb.tile([128, CH], F32R, tag="rec", bufs=3,
                          name=f"rec{b}{h}{c}")
            with nc.allow_low_precision(reason="f32r reciprocal (12-bit) ok"):
                nc.vector.reciprocal(rec[64:65, :], po[64:65, :])
            bcp = ps.tile([128, CH], F32, tag="bcx", bufs=2,
                          name=f"bcp{b}{h}{c}")
            nc.tensor.matmul(
                bcp[:, :], ones_sb[64:65, 0:128], rec[64:65, :],
                start=True, stop=True,
            )
            bc = sb.tile([128, CH], F32, tag="bc", bufs=3,
                         name=f"bc{b}{h}{c}")
            if (c + h) % 2 == 0:
                nc.vector.tensor_copy(bc[:, :], bcp[:, :])
            else:
                nc.scalar.copy(bc[:, :], bcp[:, :])
            nc.vector.tensor_tensor(
                oTz[0:64, qs: qs + CH],
                po[0:64, :], bc[0:64, :], MUL)
        return run

    def outproj_unit(b, qt, chp):
        def run():
            par = b % 2
            r0 = b * S
            pf = ps.tile([128, 512], F32, tag="bcx", bufs=2,
                         name=f"pf{b}{qt}{chp}")
            d0 = chp * 512
            nc.tensor.matmul(
                pf[:, :],
                oz[0][par][:, qt * 128:(qt + 1) * 128],
                wo0_sb[:, d0:d0 + 512],
                start=True, stop=False,
            )
            nc.tensor.matmul(
                pf[:, :],
                oz[1][par][:, qt * 128:(qt + 1) * 128],
                wo1_sb[:, d0:d0 + 512],
                start=False, stop=True,
            )
            os_ = sb.tile([128, 512], BF16, tag="os", bufs=6,
                          name=f"os{b}{qt}{chp}")
            if (qt + chp) % 2 == 0:
                nc.vector.tensor_copy(os_[:, :], pf[:, :])
            else:
                nc.scalar.copy(os_[:, :], pf[:, :])
            nc.sync.dma_start(
                out[r0 + qt * 128: r0 + (qt + 1) * 128,
                    chp * 512:(chp + 1) * 512],
                os_[:, :],
            )
        return run

    def rr(*streams):
        """Round-robin emit: streams = (list_of_units, weight) pairs."""
        idx = [0.0] * len(streams)
        pos = [0] * len(streams)
        while any(pos[i] < len(s) for i, (s, w) in enumerate(streams)):
            for i, (s, w) in enumerate(streams):
                idx[i] += w
                while idx[i] >= 1.0 and pos[i] < len(s):
                    s[pos[i]]()
                    pos[i] += 1
                    idx[i] -= 1.0

    # prologue: QKV(0)
    for cp in range(S // QKCH):
        qkv_unit(0, cp)()

    QTPC = CH // 128   # outproj token-tiles per attn chunk
    for b in range(B):
        # attention for batch b with batch b's outproj interleaved at a
        # one-chunk lag (outproj qt needs attn chunk qt//QTPC of both heads).
        attn = []
        for c in range(NC_CH):
            for h in range(2):
                attn.append(attn_unit(b, c, h))
            if c >= 1:
                for qt in range(QTPC * (c - 1), QTPC * c):
                    for chp in range(2):
                        attn.append(outproj_unit(b, qt, chp))
        streams = [(attn, 1.0)]
        if b + 1 < B:
            streams.append(([qkv_unit(b + 1, cp) for cp in range(S // QKCH)],
                            (S // QKCH) / len(attn)))
        rr(*streams)
        for qt in range(QTPC * (NC_CH - 1), QTPC * NC_CH):
            for chp in range(2):
                outproj_unit(b, qt, chp)()


_NC_CACHE = None


def _get_nc():
    global _NC_CACHE
    if _NC_CACHE is None:
        _NC_CACHE = _build_nc()
    return _NC_CACHE


def kernel(x, Wq, bq, Wk, bk, Wv, bv, Wo, bo):
    global LAST_RESULTS, LAST_IN_MAPS
    x = np.asarray(x, dtype=np.float32)
    Wq = np.asarray(Wq, dtype=np.float32)
    Wk = np.asarray(Wk, dtype=np.float32)
    Wv = np.asarray(Wv, dtype=np.float32)
    Wo = np.asarray(Wo, dtype=np.float32)
    bq = np.asarray(bq, dtype=np.float32)
    bk = np.asarray(bk, dtype=np.float32)
    bv = np.asarray(bv, dtype=np.float32)
    bo = np.asarray(bo, dtype=np.float32)

    xTb = np.ascontiguousarray(x.reshape(ROWS, D).T).astype(NPBF)

    # masks[m][p, f] = 1 if f >= 128*m + p else 0   (m = kt - NMASK*c)
    p = np.arange(128)[:, None]
    f = np.arange(CH)[None, :]
    masks = np.concatenate(
        [(f >= 128 * m + p).astype(NPBF) for m in range(NMASK)], axis=1)
    ones = np.ones((128, 128), dtype=np.float32)

    in_maps = []
    for core in range(NB):
        h0, h1 = 2 * core, 2 * core + 1
        bv_cat = np.concatenate([bv[h0], bv[h1]])            # [128]
        in_maps.append(dict(
            xT=xTb,
            wq=np.concatenate([Wq[h0], Wq[h1]], axis=1).astype(NPBF),
            wk=np.concatenate([Wk[h0], Wk[h1]], axis=1).astype(NPBF),
            wv=np.concatenate([Wv[h0], Wv[h1]], axis=1).astype(NPBF),
            wo0=np.concatenate([Wo[128 * core: 128 * core + 64],
                                np.zeros((64, D), np.float32)]).astype(NPBF),
            wo1=np.concatenate([Wo[128 * core + 64: 128 * core + 128],
                                np.zeros((64, D), np.float32)]).astype(NPBF),
            bq=np.concatenate([bq[h0], bq[h1]])[:, None].astype(np.float32),
            bk=np.concatenate([bk[h0], bk[h1]])[:, None].astype(np.float32),
            bv=np.broadcast_to(bv_cat[None, :], (128, 128)).astype(NPBF),
            masks=masks, ones=ones,
        ))

    LAST_IN_MAPS = in_maps
    nc = _get_nc()
    kwargs = {}
    if TRACE:
        kwargs = dict(trace=True, trace_cores=list(range(NB)))
    res = run_bass_kernel_spmd(nc, in_maps, core_ids=list(range(NB)), **kwargs)
    LAST_RESULTS = res

    acc = res.results[0]["out"].astype(np.float32)
    for core in range(1, NB):
        acc = acc + res.results[core]["out"].astype(np.float32)
    acc += bo[None, :]
    return acc.reshape(B, S, D)
